# revision 1
# baseline (speedup 1.0000x reference)
"""Trainium2 kernel for nn_H100SmartEmbedding (embedding_lookup).

Output [131072, 768] f32: cols 0:128 price_w[0] (const), 128:256 size_w[0]
(const), 256:384 exchange_w[i%3], 384:512 pair_w[i%7], 512:640 level_w[i%15],
640:768 time_w[i%31].  Rows repeat with period lcm(3,7,15,31)=3255.

Each of the 8 cores covers 16384 output rows.  The core builds one period
block (3328 rows, padded to 128 partitions x 26 rows) in SBUF from the tiny
tables using one-hot matmuls on the PE (block-diagonal table matrix as the
moving operand, per-chunk one-hot selectors as the stationary operand), then
replicates it to DRAM with large contiguous DMA writes.  Total HBM traffic
per core ~= 48.2 MB written + <1 MB read, which is the memory roofline.
"""

import sys

if "/opt/trn_rl_repo" not in sys.path:
    sys.path.insert(0, "/opt/trn_rl_repo")

import numpy as np

N = 131072
D = 768
E = 128  # embed per type
PERIOD = 3255  # lcm(3, 7, 15, 31)
NCORES = 8
RPC = N // NCORES  # 16384 rows per core
CHUNK = 26  # rows per SBUF partition
BROWS = CHUNK * 128  # 3328 staged rows (period + padding)
NREPS = RPC // PERIOD  # 5 full repetitions; remainder via tail writes
TAIL = RPC - NREPS * PERIOD  # 109

# packed bf16 input tensor free-dim layout (per partition, bf16 elems).
# Contraction row counts padded to multiples of 4 (zero rows) for FWL.
KC, K3, K7, K15, K31 = 8, 12, 24, 48, 96
CS_OFF = 0        # const rhs  [8, 256] block-diag (price|size) hi/mid/lo
ONES_OFF = 256    # const lhsT [8, 128] ones on first 6 rows
T3_OFF = 384      # exch rhs   [12, 128]  (3 comps x 3 rows + pad)
T7_OFF = 512      # pair rhs   [24, 128]
T15_OFF = 640     # level rhs  [48, 128]
T31_OFF = 768     # time rhs   [96, 128]
OH3_OFF = 896     # exch one-hots  [12, 26*128]
OH7_OFF = 4224    # pair one-hots  [24, 26*128]
OH15_OFF = 7552   # level one-hots [48, 26*128]
OH31_OFF = 10880  # time one-hots  [96, 26*128]
PK_F = 14208
# PSUM: two 4-bank tensors accA [128,2048] and accB [128,2048]
PS_CONST = 0      # accA [128, 256]
PS_EXCH = 256     # accA 3 slots x 128
PS_PAIR = 640     # accA 7 slots
PS_TIME = 1536    # accA 4 rotating slots
PS_LEVEL = 0      # accB 15 slots

TRACE = False
LAST_EXEC_NS = None
LAST_RESULT = None

_nc_cache = {}


def _ensure_ntff_hook():
    """The agent image's antenv package lacks axon_hooks, so the boot shim
    never registers the NTFF profile hook and trace=True crashes on import.
    Recreate the module + ctypes hook here (same recipe as trn_boot.py)."""
    import types
    import ctypes
    import contextlib

    try:
        from antenv.axon_hooks import get_axon_ntff_profile_hook  # noqa: F401
        return
    except ImportError:
        pass

    import antenv

    mod = types.ModuleType("antenv.axon_hooks")
    mod._hook = None

    def set_axon_ntff_profile_hook(h):
        mod._hook = h

    def get_axon_ntff_profile_hook():
        return mod._hook

    mod.set_axon_ntff_profile_hook = set_axon_ntff_profile_hook
    mod.get_axon_ntff_profile_hook = get_axon_ntff_profile_hook
    sys.modules["antenv.axon_hooks"] = mod
    antenv.axon_hooks = mod

    so_path = "/opt/axon/libaxon_pjrt.so"
    try:
        lib = ctypes.CDLL(so_path)
    except OSError:
        return
    if not hasattr(lib, "axon_start_nrt_profile"):
        return
    lib.axon_start_nrt_profile.argtypes = [
        ctypes.POINTER(ctypes.c_int64),
        ctypes.c_size_t,
    ]
    lib.axon_start_nrt_profile.restype = ctypes.c_int64
    lib.axon_stop_nrt_profile.argtypes = [ctypes.c_char_p]
    lib.axon_stop_nrt_profile.restype = ctypes.c_int64

    @contextlib.contextmanager
    def _hook(output_dir, device_ids):
        import jax

        jax.devices()
        if device_ids:
            ids = (ctypes.c_int64 * len(device_ids))(*device_ids)
            rc = lib.axon_start_nrt_profile(ids, len(device_ids))
        else:
            rc = lib.axon_start_nrt_profile(None, 0)
        if rc != 0:
            raise RuntimeError(f"axon_start_nrt_profile rc={rc}")
        try:
            yield
        finally:
            n = lib.axon_stop_nrt_profile(str(output_dir).encode())
            if n < 0:
                raise RuntimeError(f"axon_stop_nrt_profile rc={n}")
            print(f"profile: {n} file(s) written to {output_dir}",
                  file=sys.stderr)

    set_axon_ntff_profile_hook(_hook)


def _build_nc():
    if "nc" in _nc_cache:
        return _nc_cache["nc"]
    import concourse.bass as bass
    import concourse.mybir as mybir

    f32 = mybir.dt.float32
    bf16 = mybir.dt.bfloat16
    nc = bass.Bass()
    pk_d = nc.declare_dram_parameter("pk", [128, PK_F], bf16, isOutput=False)
    out = nc.declare_dram_parameter("out", [RPC, D], f32, isOutput=True)

    pk = nc.sbuf_tensor("pk_sb", [128, PK_F], bf16).__enter__()
    b_sb = nc.sbuf_tensor("b_sb", [128, CHUNK * D], f32).__enter__()
    # PSUM bank discipline: a bank is never read while the PE can still
    # write it (concurrent same-bank access aborts execution on TRN2 here).
    # accS: const/exch/pair, fully written (11 mms) before any copy.
    # Bank ownership per READER too (PSUM banks are single-port; any
    # same-cycle same-bank access from two engines is fatal): bank0 const
    # (vector only), banks 1-3 exch+pair (scalar only).
    # accLT: level+time recomputed per chunk side by side (one 256-col copy
    # per chunk); chunk-pairs alternate banks, vec_sem backpressure.
    accS = nc.psum_tensor("accS", [128, 2048], f32).__enter__()
    accLT = nc.psum_tensor("accLT", [128, 1024], f32).__enter__()

    GROUPS = [(0, 4), (4, 4), (8, 4), (12, 4), (16, 4), (20, 4), (24, 2)]
    SPLIT = OH15_OFF  # pk load split: static needs cols < SPLIT

    def slot(q):
        return ((q // 2) % 2) * 512 + (q % 2) * 256

    with (nc.Block() as block,
          nc.semaphore("dma_sem") as dma_sem,
          nc.semaphore("pe_sem") as pe_sem,
          nc.semaphore("vec_sem") as vec_sem):

        @block.sync
        def _(sync):
            n = 0
            sync.dma_start(out=pk[:], in_=pk_d[:]).then_inc(dma_sem, 16)
            n += 16
            # rep 0: interleaved chunk-group writes (row j = 26*p + q); the
            # 128-partition shape keeps all 16 SDMA engines loaded.  Rows
            # >= PERIOD carry wrap-correct content identical to what rep 1
            # rewrites there, so no ordering between DMAs is needed.
            for q0, g in GROUPS:
                sync.wait_ge(vec_sem, q0 + g)
                dst = bass.AP(out, q0 * D, [[CHUNK * D, 128], [1, g * D]])
                sync.dma_start(out=dst,
                               in_=b_sb[:, q0 * D:(q0 + g) * D]).then_inc(
                                   dma_sem, 16)
                n += 16
            # reps 1..4: full-block contiguous writes (128 partitions)
            for k in range(1, NREPS):
                base = k * PERIOD
                sync.dma_start(out=out[base:base + BROWS, :],
                               in_=b_sb[:]).then_inc(dma_sem, 16)
                n += 16
            # tail rows 16275..16383 (overlap with rep 4 is identical bytes)
            tbase = NREPS * PERIOD
            fp = TAIL // CHUNK  # 4
            rem = TAIL - fp * CHUNK  # 5
            sync.dma_start(out=out[tbase:tbase + fp * CHUNK, :],
                           in_=b_sb[0:fp, :]).then_inc(dma_sem, 16)
            n += 16
            sync.dma_start(out=out[tbase + fp * CHUNK:RPC, :],
                           in_=b_sb[fp:fp + 1, 0:rem * D]).then_inc(
                               dma_sem, 16)
            n += 16
            sync.wait_ge(dma_sem, n)

        @block.tensor
        def _(tensor):
            tensor.wait_ge(dma_sem, 16)
            # pe_sem: const=1, exch q->2+q, pair q->5+q,
            #         level q->12+2q, time q->13+2q
            tensor.matmul(accS[:, 0:256],
                          pk[0:KC, ONES_OFF:ONES_OFF + E],
                          pk[0:KC, CS_OFF:CS_OFF + 256],
                          skip_group_check=True).then_inc(pe_sem)
            for q in range(3):
                tensor.matmul(accS[:, 512 + q * E:512 + (q + 1) * E],
                              pk[0:K3, OH3_OFF + q * E:OH3_OFF + (q + 1) * E],
                              pk[0:K3, T3_OFF:T3_OFF + E],
                              skip_group_check=True).then_inc(pe_sem)
            for q in range(7):
                tensor.matmul(accS[:, 1024 + q * E:1024 + (q + 1) * E],
                              pk[0:K7, OH7_OFF + q * E:OH7_OFF + (q + 1) * E],
                              pk[0:K7, T7_OFF:T7_OFF + E],
                              skip_group_check=True).then_inc(pe_sem)
            for q in range(CHUNK):
                if q >= 4 and q % 2 == 0:
                    tensor.wait_ge(vec_sem, q - 2)
                s = slot(q)
                tensor.matmul(accLT[:, s:s + E],
                              pk[0:K15, OH15_OFF + q * E:OH15_OFF + (q + 1) * E],
                              pk[0:K15, T15_OFF:T15_OFF + E],
                              skip_group_check=True).then_inc(pe_sem)
                tensor.matmul(accLT[:, s + E:s + 256],
                              pk[0:K31, OH31_OFF + q * E:OH31_OFF + (q + 1) * E],
                              pk[0:K31, T31_OFF:T31_OFF + E],
                              skip_group_check=True).then_inc(pe_sem)

        @block.vector
        def _(vector):
            for q in range(CHUNK):
                if q == 0:
                    vector.wait_ge(pe_sem, 11)
                vector.tensor_copy(b_sb[:, q * D:q * D + 256],
                                   accS[:, 0:256])
                vector.tensor_copy(b_sb[:, q * D + 256:q * D + 384],
                                   accS[:, 512 + (q % 3) * E:
                                         512 + (q % 3 + 1) * E])
                vector.tensor_copy(b_sb[:, q * D + 384:q * D + 512],
                                   accS[:, 1024 + (q % 7) * E:
                                         1024 + (q % 7 + 1) * E])
                vector.wait_ge(pe_sem, 13 + 2 * q)
                s = slot(q)
                vector.tensor_copy(b_sb[:, q * D + 512:(q + 1) * D],
                                   accLT[:, s:s + 256]).then_inc(vec_sem)

    _nc_cache["nc"] = nc
    return nc


def _split3(v):
    """Exact truncation split of fp32 into 3 bit-disjoint bf16 parts."""
    v = np.ascontiguousarray(v, np.float32)
    hi = (v.view(np.uint32) & 0xFFFF0000).view(np.float32)
    r1 = v - hi
    mid = (r1.view(np.uint32) & 0xFFFF0000).view(np.float32)
    lo = r1 - mid
    return hi, mid, lo


def _onehot3(k, phi):
    """[3k, 26*128] selector: row c*k+i, col q*128+p hot iff
    (phi + 26p + q) % k == i, replicated for the 3 bf16 components."""
    arr = np.zeros((3 * k, CHUNK * E), np.float32)
    p = np.arange(E)
    for q in range(CHUNK):
        idx = (phi + CHUNK * p + q) % k
        for c in range(3):
            arr[c * k + idx, q * E + p] = 1.0
    return arr


def _core_inputs(c, price_w, size_w, exchange_w, pair_w, level_w, time_w):
    import ml_dtypes

    phi = (c * RPC) % PERIOD
    pk = np.zeros((128, PK_F), np.float32)
    ph = _split3(price_w[0])
    sh = _split3(size_w[0])
    for comp in range(3):
        pk[2 * comp + 0, CS_OFF:CS_OFF + E] = ph[comp]
        pk[2 * comp + 1, CS_OFF + E:CS_OFF + 256] = sh[comp]
    pk[0:6, ONES_OFF:ONES_OFF + E] = 1.0
    for tbl, k, off, ohoff in ((exchange_w[:3], 3, T3_OFF, OH3_OFF),
                               (pair_w[:7], 7, T7_OFF, OH7_OFF),
                               (level_w[:15], 15, T15_OFF, OH15_OFF),
                               (time_w[:31], 31, T31_OFF, OH31_OFF)):
        h, m, lo = _split3(tbl)
        pk[0:k, off:off + E] = h
        pk[k:2 * k, off:off + E] = m
        pk[2 * k:3 * k, off:off + E] = lo
        pk[0:3 * k, ohoff:ohoff + CHUNK * E] = _onehot3(k, phi)
    return {"pk": pk.astype(ml_dtypes.bfloat16)}


def kernel(price_w, size_w, exchange_w, pair_w, level_w, time_w,
           num_features=N):
    global LAST_EXEC_NS, LAST_RESULT
    assert int(num_features) == N

    from concourse.bass_utils import run_bass_kernel_spmd

    args = [np.asarray(x, np.float32) for x in
            (price_w, size_w, exchange_w, pair_w, level_w, time_w)]
    in_maps = [_core_inputs(c, *args) for c in range(NCORES)]

    if TRACE:
        _ensure_ntff_hook()
    nc = _build_nc()
    res = None
    for attempt in range(3):
        try:
            res = run_bass_kernel_spmd(nc, in_maps, list(range(NCORES)),
                                       trace=TRACE)
            break
        except Exception:
            if attempt == 2:
                raise
    LAST_EXEC_NS = res.exec_time_ns
    LAST_RESULT = res
    return np.concatenate([res.results[c]["out"] for c in range(NCORES)],
                          axis=0)

